# revision 54
# baseline (speedup 1.0000x reference)
"""CQAttention Bass kernel for TRN2, 8 NeuronCores, batch-parallel, fp8 PE path.

Problem shapes (hardcoded): context [16,128,2048] f32, query [16,128,512] f32,
w [384] f32 -> out [16,512,2048] f32.

Math per batch (D=128, C=2048, Q=512):
  s[c,q]  = bias_c[c] + bias_q[q] + sum_d ctx[d,c]*wcq[d]*qry[d,q]
  s1      = softmax_c(s)            (bias_q is constant along c -> cancels)
  aT[d,c] = sum_q s1[c,q] qry[d,q]
  t[q,d]  = sum_c s1[c,q] ctx[d,c]
  b2T     = sum_q t2[q,d] s1[c,q]   (assoc: s1(s1^T ctx^T), avoids [C,C])
  out     = [ctxT; aT; ctxT*aT; ctxT*b2T]   ([4D, C] per batch)

Device strategy (all matmuls fp8e4m3 DoubleRow = 0.5 PE-cycles/row):
  - host pre-packs fp8 operand layouts: ctx d-split [64,2,C] (s lhsT),
    qryW2 = qry*wcq+wc d-split [64,2,Q] (s rhs; folds bias_c into the rhs),
    ctxC1 [128,16,132] = ctx^T c-tiled + a trailing 1/SA column, qT [128,4,128]
  - s-matmul -> psum f32 [c-tile, q]; ACT exp(s - 2) -> E fp8 SBUF
    (bias -2 keeps exp below fp8e4m3 max 240; softmax-invariant)
  - ET via PE fp8 transposes: pairs of q-tiles land in the two aligned
    step-2 lanes of one psum region; a single bitcast-uint16 DVE copy
    drains both at the 2x_1p rate (junk odd bytes ride along)
  - t-matmul: lhsT = E c-tile pairs, rhs = ctxC1; the ones column makes
    column 128 of t equal S[q] = sum_c E[c,q]  (softmax sums for free)
  - scale management (fp8 subnormal floor 2^-9): the ones column of ctxC1
    actually holds 1/SA (SA=2^8) so reciprocal(S-col) = SA/S directly;
    qryR = qT * (SA/S), t2 = t * (SA/S)^2; the epilogue descales by 1/SA
    and 1/SA^2 (fused into the psum drains)
  - a/b2 DoubleRow over q-tile pairs; rhs = stride-2 fp8 view of ET
  - epilogue per 512-chunk: aT store (DVE tensor_scalar * 2^-10),
    ctx*a / ctx*b2 (tensor_tensor vs pre-scaled ctx; DVE/Pool split)
  - out sections a, ctx*a, ctx*b2 stored bf16; host prepends the exact
    f32 ctx passthrough section and upcasts
"""

import numpy as np
import ml_dtypes

import concourse.bass as bass
import concourse.mybir as mybir
import contextlib as _cl

import concourse.tile as tile
from concourse.bass import ts, ds
from concourse.bass_utils import run_bass_kernel_spmd
from concourse.masks import make_identity

B, D, C, Q = 16, 128, 2048, 512
NCORES = 8
BPC = B // NCORES          # batches per core
NCT = C // 128             # 16 c-tiles
NQT = Q // 128             # 4 q-tiles
NCH = C // 512             # 4 c-chunks
F32 = mybir.dt.float32
BF16 = mybir.dt.bfloat16
F8 = mybir.dt.float8e4
U16 = mybir.dt.uint16
AF = mybir.ActivationFunctionType
DR = mybir.MatmulPerfMode.DoubleRow
MULT = mybir.AluOpType.mult

F8NP = ml_dtypes.float8_e4m3
BFNP = ml_dtypes.bfloat16
SA = 2.0 ** 8              # qryR scale; t2 scale is SA^2
EXPB = -2.0                # exp bias (softmax-invariant)


_SPLIT_TYPES = (
    "InstMatmult", "InstLdweights", "InstActivation", "InstTensorScalar",
    "InstTensorScalarPtr", "InstTensorScalarAffineSelect", "InstTensorTensor",
    "InstTensorCopy", "InstReciprocal", "InstMemset", "InstCopyPredicated",
    "InstBNStats", "InstStreamTranspose", "InstTensorReduce", "InstIota",
    "InstDMACopy", "InstDMA", "InstDMAGather", "InstDMAGatherAnt",
    "InstDrain",
)


def _split_multi_waits(nc, max_embedded=1):
    """walrus allows very few embedded sync-waits per compute instruction
    (AP-parameterized ops seem to have just one slot). Hoist extra waits
    into standalone event-semaphore instructions on the same engine."""
    n = 0
    for fn in nc.m.functions:
        for blk in fn.blocks:
            il = blk.instructions
            i = 0
            while i < len(il):
                inst = il[i]
                si = inst.sync_info
                if (si is not None and si.on_wait
                        and len(si.on_wait) > max_embedded
                        and type(inst).__name__ in _SPLIT_TYPES):
                    waits = list(si.on_wait)
                    extra, keep = waits[:-max_embedded], waits[-max_embedded:]
                    for k, w in enumerate(extra):
                        nop = mybir.InstEventSemaphore(
                            name=f"{inst.name}-w{k}", engine=inst.engine,
                            ins=[], outs=[])
                        nop.sync_info = mybir.SyncInfo(on_wait=[w],
                                                       on_update=[])
                        il.insert(i, nop)
                        i += 1
                        n += 1
                    inst.sync_info = mybir.SyncInfo(on_wait=keep,
                                                    on_update=si.on_update)
                i += 1
    return n


def build_kernel():
    nc = bass.Bass("TRN2", target_bir_lowering=False, debug=False,
                   num_devices=NCORES)
    # packed inputs: one 64-partition and one 128-partition byte tensor
    # pk64[b]  = qw [64,2,Q] fp8 (1024B) ++ ctx_s [64,NCT,2,128] fp8 (4096B,
    #            c-tile-major so an early prefix DMA covers the first tiles)
    # pk128[b] = ctxc [128,16,132] fp8 (2112B) ++ qt [128,4,128] fp8 (512B)
    #            ++ ctx8 [128,C] fp8 (2048B)
    U8 = mybir.dt.uint8
    pk64_ext = nc.dram_tensor("pk64", [BPC, 64, 5120], U8,
                              kind="ExternalInput").ap()
    pk128_ext = nc.dram_tensor("pk128", [BPC, 128, 4672], U8,
                               kind="ExternalInput").ap()
    out_ext = nc.dram_tensor("out", [BPC, 3, 128, C], BF16,
                             kind="ExternalOutput").ap()

    with tile.TileContext(nc) as tc:
        with _cl.ExitStack() as ex:
            singles = ex.enter_context(tc.tile_pool(name="singles", bufs=1))
            bb = ex.enter_context(tc.tile_pool(name="bb", bufs=2))
            stg = ex.enter_context(tc.tile_pool(name="stg", bufs=6))
            ps_s = ex.enter_context(
                tc.tile_pool(name="ps_s", bufs=2, space="PSUM"))
            tr_pool = ex.enter_context(
                tc.tile_pool(name="tr", bufs=2, space="PSUM"))
            ab_pool = ex.enter_context(
                tc.tile_pool(name="ab", bufs=2, space="PSUM"))

            # ---- constants ----
            ident8 = singles.tile([128, 128], F8)
            make_identity(nc, ident8)
            bias_exp = singles.tile([128, 1], F32)
            nc.gpsimd.memset(bias_exp, EXPB)
            # PE warm-up: keeps the PE p-state clock ramped while the first
            # DMA loads land
            p_w = tr_pool.tile([128, 2, 512, 2], F8, tag="tr")
            for k in range(16):
                nc.tensor.transpose(
                    p_w[:, k % 2, ts(k % 4, 128), 0:1], ident8, ident8)
            ident_chk = singles.tile([128, 128], F8)
            nc.vector.tensor_copy(ident_chk, p_w[:, 0, 0:128, 0:1]
                                  .rearrange("p c o -> p (c o)"))

            U8 = mybir.dt.uint8
            st = {}   # per-batch tile state

            def emit_load(b):
                prio = tc.high_priority() if b == 0 else _cl.nullcontext()
                pk64_sb = bb.tile([64, 5120], U8, tag="pk64")
                pk128_sb = bb.tile([128, 4672], U8, tag="pk128")
                with prio:
                    nc.sync.dma_start(out=pk64_sb[:, 0:2048],
                                      in_=pk64_ext[b][:, 0:2048])
                    nc.sync.dma_start(out=pk64_sb[:, 2048:5120],
                                      in_=pk64_ext[b][:, 2048:5120])
                    nc.sync.dma_start(out=pk128_sb, in_=pk128_ext[b])
                d = dict(
                    ctxs=pk64_sb[:, 1024:5120].bitcast(F8).rearrange(
                        "p (j h c) -> p j h c", j=NCT, h=2),
                    qw=pk64_sb[:, 0:1024].bitcast(F8).rearrange(
                        "p (h q) -> p h q", h=2),
                    ctxc=pk128_sb[:, 0:2112].bitcast(F8).rearrange(
                        "p (j w) -> p j w", j=NCT),
                    qt=pk128_sb[:, 2112:2624].bitcast(F8).rearrange(
                        "p (j d) -> p j d", j=NQT),
                    ctx8=pk128_sb[:, 2624:4672].bitcast(F8),
                    E8=bb.tile([128, NCT, Q], F8, tag="E",
                               name=f"E8_{b}"),
                    ET=bb.tile([128, 2, NCH, 2, 512, 2], F8, tag="ET",
                               name=f"ET_{b}"),
                    t2=bb.tile([128, NQT, 128], F8, tag="t2",
                               name=f"t2_{b}"),
                    qryR=bb.tile([128, NQT, 128], F8, tag="qryR",
                                 name=f"qryR_{b}"),
                    recipA=bb.tile([128, NQT], F32, tag="recipA",
                                   name=f"recipA_{b}"),
                    sec_a=stg.tile([128, C], BF16, tag="sec_a",
                                   name=f"sec_a_{b}"),
                    sec_ca=stg.tile([128, C], BF16, tag="sec_ca",
                                    name=f"sec_ca_{b}"),
                    sec_cb=stg.tile([128, C], BF16, tag="sec_cb",
                                    name=f"sec_cb_{b}"),
                )
                st[b] = d

            def emit_sexp(b, g):
                """s-matmuls + exp for c-tiles 2g, 2g+1."""
                d = st[b]
                p_s = ps_s.tile([128, 2, Q], F32, tag="s")
                for jj in range(2):
                    nc.tensor.matmul(
                        p_s[:, jj, :],
                        lhsT=d["ctxs"][:, 2 * g + jj],
                        rhs=d["qw"], start=True, stop=True, perf_mode=DR)
                nc.scalar.activation(
                    d["E8"][:, ds(2 * g, 2), :], p_s, AF.Exp,
                    bias=bias_exp, scale=1.0)

            def emit_tr(b, jch):
                """transposes + uint16 copies for chunk jch (both jqp)."""
                d = st[b]
                tail = b == BPC - 1 and jch >= NCH - 3
                for jqp in range(2):
                    p_tr = tr_pool.tile([128, 2, 512, 2], F8, tag="tr",
                                        name=f"ptr_{jqp}")
                    for kt in range(2):
                        jq = 2 * jqp + kt
                        for j4 in range(4):
                            jc = 4 * jch + j4
                            nc.tensor.transpose(
                                p_tr[:, kt, ds(128 * j4, 128), 0:1],
                                d["E8"][:, jc, ts(jq, 128)], ident8)
                    if tail and jch < NCH - 1:
                        nc.scalar.copy(
                            d["ET"][:, jqp, jch, :, :, 0:1],
                            p_tr[:, :, :, 0:1])
                    elif tail:
                        nc.vector.tensor_copy(
                            d["ET"][:, jqp, jch].bitcast(U16),
                            p_tr.bitcast(U16))
                    else:
                        nc.vector.tensor_copy(
                            d["ET"][:, jqp, jch].bitcast(U16),
                            p_tr.bitcast(U16))

            def emit_t(b, jqh):
                """t-matmuls + recip + qryR/t2 for q-tiles 2jqh, 2jqh+1."""
                d = st[b]
                if b == BPC - 1:
                    p_t2 = ps_s.tile([128, 2, Q], F32, tag="s")
                    p_ts = [p_t2[:, 0, :], p_t2[:, 1, :]]
                else:
                    p_ta = ab_pool.tile([128, 512], F32, tag="ab")
                    p_tb = ab_pool.tile([128, 512], F32, tag="ab")
                    p_ts = [p_ta, p_tb]
                for jj in range(2):
                    jq = 2 * jqh + jj
                    p_t = p_ts[jj]
                    for i in range(8):
                        nc.tensor.matmul(
                            p_t[:, 0:129],
                            lhsT=d["E8"][:, ds(2 * i, 2), ts(jq, 128)],
                            rhs=d["ctxc"][:, ds(2 * i, 2), 0:129],
                            start=(i == 0), stop=(i == 7), perf_mode=DR)
                    # ones column holds 1/SA -> reciprocal gives SA / S
                    nc.vector.reciprocal(d["recipA"][:, jq:jq + 1],
                                         p_t[:, 128:129])
                    nc.gpsimd.tensor_scalar_mul(
                        d["qryR"][:, jq, :], d["qt"][:, jq, :],
                        d["recipA"][:, jq:jq + 1])
                    nc.vector.tensor_scalar(
                        out=d["t2"][:, jq, :], in0=p_t[:, 0:128],
                        scalar1=d["recipA"][:, jq:jq + 1],
                        scalar2=d["recipA"][:, jq:jq + 1],
                        op0=MULT, op1=MULT)

            def emit_ab(b, jch):
                """a/b2 matmuls + epilogue ops for chunk jch."""
                d = st[b]
                p_a = ab_pool.tile([128, 512], F32, tag="ab")
                if b == BPC - 1:
                    p_bw = ps_s.tile([128, 2, Q], F32, tag="s")
                    p_b = p_bw[:, 0, :]
                else:
                    p_b = ab_pool.tile([128, 512], F32, tag="ab")
                for jqp in range(2):
                    rhs = d["ET"][:, jqp, jch, :, :, 0:1]
                    nc.tensor.matmul(
                        p_a, lhsT=d["qryR"][:, ds(2 * jqp, 2), :], rhs=rhs,
                        start=(jqp == 0), stop=(jqp == 1), perf_mode=DR)
                for jqp in range(2):
                    rhs = d["ET"][:, jqp, jch, :, :, 0:1]
                    nc.tensor.matmul(
                        p_b, lhsT=d["t2"][:, ds(2 * jqp, 2), :], rhs=rhs,
                        start=(jqp == 0), stop=(jqp == 1), perf_mode=DR)
                sl = ts(jch, 512)
                # section a: descale by 1/SA (ACT helps on the tail batch)
                if b == BPC - 1:
                    nc.scalar.mul(d["sec_a"][:, sl], p_a, 1.0 / SA)
                else:
                    nc.vector.tensor_scalar_mul(d["sec_a"][:, sl], p_a,
                                                1.0 / SA)
                # section ctx*a: late chunks of the tail batch use the
                # fused psum form (no stx0 dependency -> runs parallel)
                if b == BPC - 1 and jch >= 2:
                    nc.vector.scalar_tensor_tensor(
                        out=d["sec_ca"][:, sl], in0=p_a, scalar=1.0 / SA,
                        in1=d["ctx8"][:, sl], op0=MULT, op1=MULT)
                else:
                    eng_ca = nc.vector if (b == BPC - 1 and jch % 2) \
                        else nc.gpsimd
                    eng_ca.tensor_tensor(
                        out=d["sec_ca"][:, sl], in0=d["sec_a"][:, sl],
                        in1=d["ctx8"][:, sl], op=MULT)
                # section ctx*b2 = (p_b / SA^2) * ctx  (fused psum drain)
                nc.vector.scalar_tensor_tensor(
                    out=d["sec_cb"][:, sl], in0=p_b, scalar=1.0 / SA ** 2,
                    in1=d["ctx8"][:, sl], op0=MULT, op1=MULT)

            def emit_store(b, lo, hi, pool_dma=False):
                d = st[b]
                for sec, key in ((0, "sec_a"), (1, "sec_ca"), (2, "sec_cb")):
                    eng = nc.gpsimd if (pool_dma and sec == 2) else nc.sync
                    eng.dma_start(out=out_ext[b, sec, :, lo:hi],
                                  in_=d[key][:, lo:hi])

            # ---- emission in data-readiness order (PE/ACT are in-order) ----
            emit_load(0)
            emit_load(1)
            for g in range(8):
                emit_sexp(0, g)
                if g % 2 == 1:
                    emit_tr(0, g // 2)
            emit_sexp(1, 0)
            emit_sexp(1, 1)
            emit_t(0, 0)
            emit_t(0, 1)
            emit_ab(0, 0)
            emit_sexp(1, 2)
            emit_ab(0, 1)
            emit_sexp(1, 3)
            emit_tr(1, 0)
            emit_ab(0, 2)
            emit_sexp(1, 4)
            emit_ab(0, 3)
            emit_store(0, 0, 1024)
            emit_sexp(1, 5)
            emit_tr(1, 1)
            emit_sexp(1, 6)
            emit_sexp(1, 7)
            emit_store(0, 1024, 2048)
            emit_tr(1, 2)
            emit_t(1, 0)
            emit_t(1, 1)
            emit_ab(1, 0)
            emit_ab(1, 1)
            emit_tr(1, 3)
            emit_store(1, 0, 1024)
            emit_ab(1, 2)
            emit_ab(1, 3)
            emit_store(1, 1024, 2048, pool_dma=True)
    _split_multi_waits(nc)
    return nc


_NC = None


def _prep_inputs(context, query, w):
    """Host-side sharding + fp8 packing into the two byte tensors."""
    f32 = np.float32
    context = np.ascontiguousarray(context, dtype=f32)
    query = np.ascontiguousarray(query, dtype=f32)
    w = np.ascontiguousarray(w, dtype=f32)
    wc, wcq = w[D:2 * D], w[2 * D:3 * D]

    # ctx_s[b, p, j, h, i] = ctx[b, 64h+p, 128j+i]  (c-tile-major)
    ctx_s = np.ascontiguousarray(
        context.reshape(B, 2, 64, NCT, 128).transpose(0, 2, 3, 1, 4)
    ).astype(F8NP)
    qw = np.ascontiguousarray(
        (query * wcq[None, :, None] + wc[None, :, None])
        .reshape(B, 2, 64, Q).transpose(0, 2, 1, 3)).astype(F8NP)
    pk64 = np.concatenate([
        qw.reshape(B, 64, 2 * Q).view(np.uint8),
        ctx_s.reshape(B, 64, 2 * C).view(np.uint8)], axis=2)

    # ctxc[b, p, j, 0:128] = ctx[b, :, 128j+p]; col 128 = 1/SA (S column)
    ctxc = np.zeros((B, 128, NCT, 132), dtype=F8NP)
    ctxc[:, :, :, 0:128] = (context.transpose(0, 2, 1)
                            .reshape(B, NCT, 128, D)
                            .transpose(0, 2, 1, 3)).astype(F8NP)
    ctxc[:, :, :, 128] = np.float32(1.0 / SA).astype(F8NP)
    qt = np.ascontiguousarray(
        query.transpose(0, 2, 1).reshape(B, NQT, 128, D)
        .transpose(0, 2, 1, 3)).astype(F8NP)
    ctx8 = context.astype(F8NP)
    pk128 = np.concatenate([
        ctxc.reshape(B, 128, NCT * 132).view(np.uint8),
        qt.reshape(B, 128, NQT * 128).view(np.uint8),
        ctx8.view(np.uint8)], axis=2)

    in_maps = []
    for i in range(NCORES):
        sl = slice(i * BPC, (i + 1) * BPC)
        in_maps.append({"pk64": pk64[sl], "pk128": pk128[sl]})
    return context, in_maps


def kernel(context: np.ndarray, query: np.ndarray, w: np.ndarray,
           **extra) -> np.ndarray:
    global _NC
    if _NC is None:
        _NC = build_kernel()
    context, in_maps = _prep_inputs(context, query, w)
    res = run_bass_kernel_spmd(_NC, in_maps, core_ids=list(range(NCORES)))
    dev = np.concatenate([r["out"] for r in res.results], axis=0)  # [B,3,128,C]
    out = np.empty((B, 4 * D, C), dtype=np.float32)
    out[:, 0:D, :] = context
    out[:, D:4 * D, :] = dev.astype(np.float32).reshape(B, 3 * D, C)
    return out


if __name__ == "__main__":
    rng = np.random.default_rng(0)
    out = kernel(
        context=rng.standard_normal((B, D, C), dtype=np.float32),
        query=rng.standard_normal((B, D, Q), dtype=np.float32),
        w=(rng.random(3 * D, dtype=np.float32) - 0.5) * 2 / np.sqrt(D),
    )
    print(out.shape, out.dtype)
